# revision 37
# baseline (speedup 1.0000x reference)
"""Trainium2 Bass kernel for nn_MAB (dense transformer block).

Reference (B=32, N=512, D=512, H=8, dh=64):
    q = Q @ Wq.T; k = K @ Wk.T; v = K @ Wv.T          (biases zero)
    scores = einsum("bqhd,bkhd->bhqk", q, k) / sqrt(512)
    A = softmax(scores, axis=2)                        # QUERY axis
    attn = einsum("bhqk,bkhd->bqhd", A, v)
    out = Q + attn @ Wo.T
    return out + relu(out @ W1.T) @ W2.T

Data-parallel over batch: 8 cores x 4 batches, zero collectives.

Precision/speed strategy (validated vs reference in numpy, ~4x margin
to the 2e-2 gate): matmuls run as fp8e4m3 DoubleRow (0.5 cyc/row, 2x
contraction per instruction = 4x bf16 throughput) wherever error
analysis allows.  The attention path has small magnitudes relative to
the residual stream, so q/k/v projections, attn-apply and Wo run
single-fp8.  The FFN path amplifies input error, so W1/W2 use hi+lo
fp8 weight splits (host-side, free) against hi+lo fp8 activation
splits (one extra DVE op each) - 3-term products at 0.75x bf16 cost
with ~bf16 accuracy.  Scores stay bf16 (dh=64 contraction cannot be
DoubleRow-paired: pairs must sit in the free axis, but proj outputs
put features on partitions).  Softmax-over-q normalizers fold into v
(64x fewer elements than A).  Residual adds fuse into the PSUM
evacuation ops (scalar_tensor_tensor), costing zero extra passes.

Engine placement: ACT does exp (multi-bank instructions) + q-evac;
DVE does the E row-sum accum passes (2x_2p mode on SBUF fp8) and the
hi/lo splits; Pool (otherwise idle) takes the remaining PSUM
evacuations; weights/inputs ship fp8/bf16 to halve DMA.
"""

import math
import os
import sys

import numpy as np
import ml_dtypes

sys.path.insert(0, "/opt/trn_rl_repo")

import concourse.bass as bass  # noqa: E402
import concourse.tile as tile  # noqa: E402
from concourse import bacc  # noqa: E402
from concourse import mybir  # noqa: E402
from concourse.bass_utils import run_bass_kernel_spmd  # noqa: E402

F32 = mybir.dt.float32
BF16 = mybir.dt.bfloat16
FP8 = mybir.dt.float8e4
AF = mybir.ActivationFunctionType
ALU = mybir.AluOpType
DR = mybir.MatmulPerfMode.DoubleRow

B, N, D, H = 32, 512, 512, 8
DH = D // H  # 64
NCORES = 8
BLOC = B // NCORES  # 4
SCALE = 1.0 / math.sqrt(512.0)
P = 128
KC = D // P  # 4 contraction / feature chunks
SW = 16.0  # weight pre-scale (host)
NACC = 0  # heads whose exp uses per-tile ACT accum (S on ACT not DVE)

NP_FP8 = ml_dtypes.float8_e4m3
NP_BF16 = ml_dtypes.bfloat16

_CACHE = {}


def _build_program(with_bias):
    nc = bacc.Bacc("TRN2", target_bir_lowering=False, debug=False,
                   num_devices=NCORES)

    # ---- DRAM I/O ------------------------------------------------------
    qhi_d = nc.dram_tensor("qhi", [BLOC, D, N], FP8, kind="ExternalInput").ap()
    qbf_d = nc.dram_tensor("qbf", [BLOC, D, N], BF16, kind="ExternalInput").ap()
    k8_d = nc.dram_tensor("k8", [BLOC, D, N], FP8, kind="ExternalInput").ap()
    w_names = ("wq", "wk", "wv", "wo", "w1h", "w1l", "w2h", "w2l")
    w_d = {nm: nc.dram_tensor(nm, [D, D], FP8, kind="ExternalInput").ap()
           for nm in w_names}
    b_d = {}
    if with_bias:
        for nm in ("bq", "bk", "bv16", "bo", "b116", "b2128"):
            b_d[nm] = nc.dram_tensor(nm, [D], F32, kind="ExternalInput").ap()
    outT_d = nc.dram_tensor("outT", [BLOC, D, N], F32,
                            kind="ExternalOutput").ap()

    qhi_v = qhi_d.rearrange("b (o p) t -> b p o t", p=P)
    qbf_v = qbf_d.rearrange("b (o p) t -> b p o t", p=P)
    k8_v = k8_d.rearrange("b (o p) t -> b p o t", p=P)
    outT_v = outT_d.rearrange("b (o p) t -> b p o t", p=P)
    w_v = {k: v.rearrange("(o p) n -> p o n", p=P) for k, v in w_d.items()}
    b_v = {k: v.rearrange("(o p) -> p o", p=P) for k, v in b_d.items()}

    with tile.TileContext(nc) as tc:
        with (
            tc.tile_pool(name="weights", bufs=1) as wpool,
            tc.tile_pool(name="qin", bufs=2) as qin_pool,
            tc.tile_pool(name="kin", bufs=2) as kin_pool,
            tc.tile_pool(name="projqk", bufs=2) as pqk_pool,
            tc.tile_pool(name="epool", bufs=2) as e_pool,
            tc.tile_pool(name="spool", bufs=2) as s_pool,
            tc.tile_pool(name="vpool", bufs=2) as v_pool,
            tc.tile_pool(name="opool", bufs=2) as o_pool,
            tc.tile_pool(name="hpool", bufs=2) as h_pool,
            tc.tile_pool(name="fpool", bufs=2) as f_pool,
            tc.tile_pool(name="ps2", bufs=2, space="PSUM") as ps2,
            tc.tile_pool(name="psS2", bufs=2, space="PSUM") as psS2,
        ):
            # ---- resident weights -------------------------------------
            w_sb = {}
            for nm in w_names:
                w_sb[nm] = wpool.tile([P, KC, D], FP8, tag=f"w_{nm}",
                                      name=f"w_{nm}")
            for nm in ("wq", "wk"):
                nc.sync.dma_start(out=w_sb[nm][:], in_=w_v[nm])
            b_sb = {}
            if with_bias:
                for nm in ("bq", "bk", "bv16", "bo", "b116", "b2128"):
                    b_sb[nm] = wpool.tile([P, KC], F32, tag=f"b_{nm}",
                                          name=f"b_{nm}")
                    nc.sync.dma_start(out=b_sb[nm][:], in_=b_v[nm])

            st = {}

            def emit_proj(b):
                qhi_t = qin_pool.tile([P, KC, N], FP8, tag="qhi")
                nc.sync.dma_start(out=qhi_t[:], in_=qhi_v[b])
                qbf_t = qin_pool.tile([P, KC, N], BF16, tag="qbf")
                nc.sync.dma_start(out=qbf_t[:], in_=qbf_v[b])
                k8_t = kin_pool.tile([P, KC, N], FP8, tag="k8")
                nc.sync.dma_start(out=k8_t[:], in_=k8_v[b])
                if b == 0:
                    # deferred weight loads overlap with b=0 compute
                    for nm in ("wv", "wo", "w1h", "w1l", "w2h", "w2l"):
                        nc.sync.dma_start(out=w_sb[nm][:], in_=w_v[nm])

                qh = pqk_pool.tile([P, KC, N], BF16, tag="qh")
                kh = pqk_pool.tile([P, KC, N], BF16, tag="kh")
                for mp in range(2):  # m-chunk pairs
                    psQ = ps2.tile([P, 2, N], F32, tag="ps")
                    psK = ps2.tile([P, 2, N], F32, tag="ps")
                    for i in range(2):
                        m = 2 * mp + i
                        ms = slice(m * P, (m + 1) * P)
                        for jp in range(2):
                            js = slice(2 * jp, 2 * jp + 2)
                            nc.tensor.matmul(
                                psQ[:, i, :], lhsT=w_sb["wq"][:, js, ms],
                                rhs=qhi_t[:, js, :], start=(jp == 0),
                                stop=(jp == 1), perf_mode=DR)
                        for jp in range(2):
                            js = slice(2 * jp, 2 * jp + 2)
                            nc.tensor.matmul(
                                psK[:, i, :], lhsT=w_sb["wk"][:, js, ms],
                                rhs=k8_t[:, js, :], start=(jp == 0),
                                stop=(jp == 1), perf_mode=DR)
                    if with_bias:
                        for i in range(2):
                            m = 2 * mp + i
                            nc.scalar.activation(
                                out=qh[:, m, :], in_=psQ[:, i, :],
                                func=AF.Identity, scale=1.0 / SW,
                                bias=b_sb["bq"][:, m:m + 1])
                            nc.scalar.activation(
                                out=kh[:, m, :], in_=psK[:, i, :],
                                func=AF.Identity, scale=1.0 / SW,
                                bias=b_sb["bk"][:, m:m + 1])
                    else:
                        # PSUM evacuations: split ACT/DVE (GPSIMD has no
                        # PSUM access)
                        nc.vector.tensor_scalar(
                            out=qh[:, 2 * mp:2 * mp + 2, :], in0=psQ[:],
                            scalar1=1.0 / SW, scalar2=None, op0=ALU.mult)
                        nc.vector.tensor_scalar(
                            out=kh[:, 2 * mp:2 * mp + 2, :], in0=psK[:],
                            scalar1=1.0 / SW, scalar2=None, op0=ALU.mult)
                st[b] = {"qh": qh, "kh": kh, "k8": k8_t, "qbf": qbf_t}

            def emit_scores(b, h0, h1):
                qh, kh = st[b]["qh"], st[b]["kh"]
                if h0 == 0:
                    st[b]["E"] = e_pool.tile([P, H, KC, N], FP8, tag="E",
                                             name="E")
                    st[b]["Ssum"] = s_pool.tile([P, H * KC], F32, tag="Ssum",
                                                name="Ssum")
                    st[b]["scr8d"] = s_pool.tile([P, N], FP8, tag="scr8d",
                                                 name="scr8d")
                E, Ssum, scr8d = st[b]["E"], st[b]["Ssum"], st[b]["scr8d"]

                for h in range(h0, h1):
                    m, half = h // 2, h % 2
                    hs = slice(64 * half, 64 * half + 64)
                    psS = [psS2.tile([P, 2, N], F32, tag="psS",
                                     name=f"psS{h}{g}")
                           for g in range(2)]
                    for j in range(KC):
                        js = slice(j * P, (j + 1) * P)
                        nc.tensor.matmul(
                            psS[j // 2][:, j % 2, :], lhsT=kh[hs, m, js],
                            rhs=qh[hs, m, :], start=True, stop=True)
                    if h < NACC:
                        # per-tile exp with fused row-sum accumulation (ACT)
                        for j in range(KC):
                            idx = h + H * j
                            nc.scalar.activation(
                                out=E[:, h, j, :], in_=psS[j // 2][:, j % 2, :],
                                func=AF.Exp, scale=SCALE,
                                accum_out=Ssum[:, idx:idx + 1])
                    else:
                        for g in range(2):
                            nc.scalar.activation(
                                out=E[:, h, 2 * g:2 * g + 2, :], in_=psS[g][:],
                                func=AF.Exp, scale=SCALE)
                        # row-sum accum passes: DVE only (2x_2p on SBUF;
                        # Pool rejects TensorScalarPtrReduce at ISA level)
                        for j in range(KC):
                            idx = h + H * j
                            nc.vector.tensor_scalar(
                                out=scr8d[:], in0=E[:, h, j, :], scalar1=1.0,
                                scalar2=0.0, op0=ALU.mult, op1=ALU.add,
                                accum_out=Ssum[:, idx:idx + 1])

            def emit_attn_tail(b):
                k8_t = st[b]["k8"]
                E, Ssum = st[b]["E"], st[b]["Ssum"]
                # ---- v projection (after exp frees PSUM) ---------------
                psV = [ps2.tile([P, 2, N], F32, tag="ps", name=f"psV{g}")
                       for g in range(2)]
                for tt in range(KC):
                    ts = slice(tt * P, (tt + 1) * P)
                    for jp in range(2):
                        js = slice(2 * jp, 2 * jp + 2)
                        nc.tensor.matmul(
                            psV[tt // 2][:, tt % 2, :],
                            lhsT=k8_t[:, js, ts], rhs=w_sb["wv"][:, js, :],
                            start=(jp == 0), stop=(jp == 1), perf_mode=DR)
                if with_bias:
                    # psV += 16*bv broadcast along free axis
                    bvb = s_pool.tile([P, D], F32, tag="bvb")
                    bsrc = bass.AP(tensor=b_d["bv16"].tensor,
                                   offset=b_d["bv16"].offset,
                                   ap=[[0, P], *b_d["bv16"].ap])
                    nc.sync.dma_start(out=bvb[:], in_=bsrc)
                    for g in range(2):
                        nc.vector.tensor_tensor(
                            out=psV[g][:], in0=psV[g][:],
                            in1=bvb[:].unsqueeze(1).to_broadcast((P, 2, D)),
                            op=ALU.add)

                # ---- rrec = 32/S ---------------------------------------
                rrec = s_pool.tile([P, H * KC], F32, tag="rrec")
                nc.vector.reciprocal(out=rrec[:], in_=Ssum[:])
                rr2 = s_pool.tile([P, H * KC], F32, tag="rr2")
                nc.vector.tensor_scalar(out=rr2[:], in0=rrec[:],
                                        scalar1=512.0 / SW, scalar2=None,
                                        op0=ALU.mult)

                # ---- vt8 = psV * rr2[k,chunk,head] (fp8) ---------------
                vt8 = v_pool.tile([P, KC, N], FP8, tag="vt8")
                for tt in range(KC):
                    # in0 [P, 8(h), 64(dh)]; in1 rr2 cols [8*tt:8*tt+8] bcast
                    rr_ap = rr2[:, H * tt:H * tt + H].unsqueeze(2) \
                        .to_broadcast((P, H, DH))
                    nc.vector.scalar_tensor_tensor(
                        out=vt8[:, tt, :].rearrange("p (h e) -> p h e", h=H),
                        in0=psV[tt // 2][:, tt % 2, :].rearrange(
                            "p (h e) -> p h e", h=H),
                        scalar=1.0, in1=rr_ap, op0=ALU.mult, op1=ALU.mult)

                # ---- attn apply: attnT[d,q] += E~ x vt8 ----------------
                psA = [ps2.tile([P, 2, N], F32, tag="ps", name=f"psA{g}")
                       for g in range(2)]
                for hp in range(KC):
                    for h2 in range(2):
                        h = 2 * hp + h2
                        ds = slice(64 * h2, 64 * h2 + 64)
                        if h2 == 0:
                            # DoubleRow requires dst partition base 0
                            for jp in range(2):
                                js = slice(2 * jp, 2 * jp + 2)
                                nc.tensor.matmul(
                                    psA[hp // 2][ds, hp % 2, :],
                                    lhsT=vt8[:, js, h * DH:(h + 1) * DH],
                                    rhs=E[:, h, js, :],
                                    start=(jp == 0), stop=(jp == 1),
                                    perf_mode=DR)
                        else:
                            for j in range(KC):
                                nc.tensor.matmul(
                                    psA[hp // 2][ds, hp % 2, :],
                                    lhsT=vt8[:, j, h * DH:(h + 1) * DH],
                                    rhs=E[:, h, j, :],
                                    start=(j == 0), stop=(j == KC - 1))
                at8 = v_pool.tile([P, KC, N], FP8, tag="at8")
                for g in range(2):
                    nc.scalar.activation(
                        out=at8[:, 2 * g:2 * g + 2, :], in_=psA[g][:],
                        func=AF.Copy, scale=16.0 / 512.0)
                st[b].update({"at8": at8})

            def emit_ffn_head(b):
                at8, qbf_t = st[b]["at8"], st[b]["qbf"]
                out_f = o_pool.tile([P, KC, N], F32, tag="out_f")
                o8h = o_pool.tile([P, KC, N], FP8, tag="o8h")
                o8l = o_pool.tile([P, KC, N], FP8, tag="o8l")
                st[b].update({"out_f": out_f, "o8h": o8h, "o8l": o8l})
                for mp in range(2):
                    psO = ps2.tile([P, 2, N], F32, tag="ps", name=f"psO{mp}")
                    for i in range(2):
                        m = 2 * mp + i
                        ms = slice(m * P, (m + 1) * P)
                        for jp in range(2):
                            js = slice(2 * jp, 2 * jp + 2)
                            nc.tensor.matmul(
                                psO[:, i, :], lhsT=w_sb["wo"][:, js, ms],
                                rhs=at8[:, js, :], start=(jp == 0),
                                stop=(jp == 1), perf_mode=DR)
                    if with_bias:
                        for i in range(2):
                            m = 2 * mp + i
                            nc.vector.tensor_scalar(
                                out=psO[:, i, :], in0=psO[:, i, :],
                                scalar1=b_sb["bo"][:, m:m + 1], scalar2=None,
                                op0=ALU.add)
                    mslc = slice(2 * mp, 2 * mp + 2)
                    # out = psO/(16*16) + Q  (residual fused into evac)
                    nc.vector.scalar_tensor_tensor(
                        out=out_f[:, mslc, :], in0=psO[:],
                        scalar=1.0 / (16.0 * SW), in1=qbf_t[:, mslc, :],
                        op0=ALU.mult, op1=ALU.add)
                    # hi/lo splits on Pool; jp-outer W-loops keep them off
                    # the critical path
                    nc.gpsimd.tensor_scalar(
                        out=o8h[:, mslc, :], in0=out_f[:, mslc, :],
                        scalar1=1.0, scalar2=None, op0=ALU.mult)
                    nc.gpsimd.tensor_tensor(
                        out=o8l[:, mslc, :], in0=out_f[:, mslc, :],
                        in1=o8h[:, mslc, :], op=ALU.subtract)

            def emit_ffn_tail(b):
                out_f = st[b]["out_f"]
                o8h, o8l = st[b]["o8h"], st[b]["o8l"]
                # ---- h1 = relu(out @ W1.T): 3-term ---------------------
                h8h = h_pool.tile([P, KC, N], FP8, tag="h8h")
                h8l = h_pool.tile([P, KC, N], FP8, tag="h8l")
                for mp in range(2):
                    psH = ps2.tile([P, 2, N], F32, tag="ps", name=f"psH{mp}")
                    for i in range(2):
                        m = 2 * mp + i
                        ms = slice(m * P, (m + 1) * P)
                        terms = [("w1h", o8h), ("w1l", o8h), ("w1h", o8l)]
                        # jp-outer: the jp=0 half only needs the first two
                        # chunks of o8h/o8l, so W1 starts before the second
                        # evac/split half lands
                        for jp in range(2):
                            js = slice(2 * jp, 2 * jp + 2)
                            for ti, (wn, rt) in enumerate(terms):
                                nc.tensor.matmul(
                                    psH[:, i, :], lhsT=w_sb[wn][:, js, ms],
                                    rhs=rt[:, js, :],
                                    start=(ti == 0 and jp == 0),
                                    stop=(ti == 2 and jp == 1), perf_mode=DR)
                    # r = relu(psH [+ 16*b1]) ; psH = 16*h1pre
                    # r = relu(psH)/2 = 8*h1  (0.5 folded into ACT scale;
                    # relu(x*0.5) == relu(x)*0.5)
                    r = h_pool.tile([P, 2, N], F32, tag="r", name=f"r{mp}")
                    if with_bias:
                        for i in range(2):
                            m = 2 * mp + i
                            nc.scalar.activation(
                                out=r[:, i, :], in_=psH[:, i, :],
                                func=AF.Relu, bias=b_sb["b116"][:, m:m + 1],
                                scale=0.5)
                    else:
                        nc.scalar.activation(
                            out=r[:], in_=psH[:], func=AF.Relu, scale=0.5)
                    mslc = slice(2 * mp, 2 * mp + 2)
                    nc.gpsimd.tensor_scalar(
                        out=h8h[:, mslc, :], in0=r[:], scalar1=1.0,
                        scalar2=None, op0=ALU.mult)
                    nc.gpsimd.tensor_tensor(
                        out=h8l[:, mslc, :], in0=r[:],
                        in1=h8h[:, mslc, :], op=ALU.subtract)

                # ---- fin = out + (8*h1) @ (16*W2.T) / 128 --------------
                fin = f_pool.tile([P, KC, N], F32, tag="fin")
                for mp in range(2):
                    psF = ps2.tile([P, 2, N], F32, tag="ps", name=f"psF{mp}")
                    for i in range(2):
                        m = 2 * mp + i
                        ms = slice(m * P, (m + 1) * P)
                        terms = [("w2h", h8h), ("w2l", h8h), ("w2h", h8l)]
                        for jp in range(2):
                            js = slice(2 * jp, 2 * jp + 2)
                            for ti, (wn, rt) in enumerate(terms):
                                nc.tensor.matmul(
                                    psF[:, i, :], lhsT=w_sb[wn][:, js, ms],
                                    rhs=rt[:, js, :],
                                    start=(ti == 0 and jp == 0),
                                    stop=(ti == 2 and jp == 1), perf_mode=DR)
                    if with_bias:
                        for i in range(2):
                            m = 2 * mp + i
                            nc.vector.tensor_scalar(
                                out=psF[:, i, :], in0=psF[:, i, :],
                                scalar1=b_sb["b2128"][:, m:m + 1],
                                scalar2=None, op0=ALU.add)
                    mslc = slice(2 * mp, 2 * mp + 2)
                    nc.vector.scalar_tensor_tensor(
                        out=fin[:, mslc, :], in0=psF[:],
                        scalar=1.0 / 128.0, in1=out_f[:, mslc, :],
                        op0=ALU.mult, op1=ALU.add)
                    # ship each half as soon as it lands
                    nc.sync.dma_start(out=outT_v[b][:, mslc, :],
                                      in_=fin[:, mslc, :])
                del st[b]

            # software pipeline, finely interleaved so no engine's in-order
            # queue head-of-line blocks independent work of other batches:
            # proj(b) || attn(b-1) || ffn(b-2)
            for step in range(BLOC + 2):
                ba, bp, bf = step - 1, step, step - 2
                if 0 <= ba < BLOC:
                    emit_scores(ba, 0, 4)
                if bp < BLOC:
                    emit_proj(bp)
                if 0 <= ba < BLOC:
                    emit_scores(ba, 4, 8)
                if bf >= 0:
                    emit_ffn_head(bf)
                if 0 <= ba < BLOC:
                    emit_attn_tail(ba)
                if bf >= 0:
                    emit_ffn_tail(bf)

    nc.compile()
    return nc


def kernel(Q, K, Wq, bq, Wk, bk, Wv, bv, Wo, bo, W1, b1, W2, b2):
    Q = np.asarray(Q, dtype=np.float32)
    K = np.asarray(K, dtype=np.float32)

    biases = {nm: np.asarray(v, np.float32) for nm, v in
              (("bq", bq), ("bk", bk), ("bv", bv),
               ("bo", bo), ("b1", b1), ("b2", b2))}
    with_bias = any(np.any(v) for v in biases.values())

    key = ("nc", with_bias)
    if key not in _CACHE:
        _CACHE[key] = _build_program(with_bias)
    nc = _CACHE[key]

    def hilo(w):
        ws = np.ascontiguousarray(np.asarray(w, np.float32).T) * SW
        hi = ws.astype(NP_FP8)
        lo = (ws - hi.astype(np.float32)).astype(NP_FP8)
        return hi, lo

    common = {}
    for nm, w in (("wq", Wq), ("wk", Wk), ("wv", Wv), ("wo", Wo)):
        ws = np.ascontiguousarray(np.asarray(w, np.float32).T) * SW
        common[nm] = ws.astype(NP_FP8)
    common["w1h"], common["w1l"] = hilo(W1)
    common["w2h"], common["w2l"] = hilo(W2)
    if with_bias:
        common["bq"] = biases["bq"]
        common["bk"] = biases["bk"]
        common["bv16"] = biases["bv"] * SW
        common["bo"] = biases["bo"] * (SW * 16.0)  # psO holds 256*out
        common["b116"] = biases["b1"] * 8.0  # relu((16*h+16*b1)*0.5)
        common["b2128"] = biases["b2"] * 128.0

    in_maps = []
    for c in range(NCORES):
        sl = slice(c * BLOC, (c + 1) * BLOC)
        qT = np.ascontiguousarray(Q[sl].transpose(0, 2, 1))
        kT = np.ascontiguousarray(K[sl].transpose(0, 2, 1))
        in_maps.append({
            "qhi": qT.astype(NP_FP8),
            "qbf": qT.astype(NP_BF16),
            "k8": kT.astype(NP_FP8),
            **common,
        })

    trace = bool(int(os.environ.get("KERNEL_TRACE", "0")))
    res = run_bass_kernel_spmd(nc, in_maps, core_ids=list(range(NCORES)),
                               trace=trace)
    if trace and res.exec_time_ns is not None:
        print(f"HW exec time: {res.exec_time_ns} ns")

    out = np.empty((B, N, D), np.float32)
    for c in range(NCORES):
        out[c * BLOC:(c + 1) * BLOC] = res.results[c]["outT"].transpose(0, 2, 1)
    return out
